# revision 56
# baseline (speedup 1.0000x reference)
"""CrossGSA fused attention kernel for 8x Trainium2 NeuronCores.

Sharding: each core owns one batch (b = core//4) and a 576-query-token slice
(qs = core%4) of that batch, across ALL 8 heads.  k/v are computed full per
core (per-batch); the mask is the dominant traffic, pre-cast to fp8e4m3 and
pre-tiled on host so each core streams it exactly once in [64, 2, h, q]
DoubleRow order.

Device layouts are transposed ([channel, token]) end-to-end:
  - projections run in bf16; the rotary pair-swap is folded into a second
    set of host-swapped (and sign-flipped) weight matrices, so rotary is
    three DVE tensor-tensor ops straight out of PSUM,
  - S^T accumulates the mask via an fp8 DoubleRow identity-matmul injection
    (half PE cost of the bf16 identity trick), then adds q.k^T over the
    head dim (K=32, 4-way tile-position packed),
  - exp() runs on the scalar engine straight out of PSUM, emitting fp8,
  - attn@v and the softmax denominators are fp8 matmuls (the ISA rejects
    DoubleRow with column tile packing, so these run at bf16 rate); the
    denominator stationary is a 32-wide ones block so the key-sum lands
    broadcast across each head's 32 partitions and reciprocal reads PSUM
    directly (no compact/broadcast round-trip),
  - the depthwise 5x5 lepe conv runs as bf16 4x-mode DVE taps,
  - layernorm stats use fp8-ones matmuls over bf16 o2; mean/rstd broadcast
    back via K=1 bf16 matmuls.
The per-core [256, 576] transposed outputs are gathered and untransposed on
host.
"""

import numpy as np
import ml_dtypes

import concourse.bass as bass
import concourse.mybir as mybir
import concourse.tile as tile
from concourse import bacc, bass_utils

F32 = mybir.dt.float32
BF16 = mybir.dt.bfloat16
F8 = mybir.dt.float8e4
AF = mybir.ActivationFunctionType
ALU = mybir.AluOpType
DR = mybir.MatmulPerfMode.DoubleRow

B, H, W, C = 2, 48, 48, 256
NH, HD = 8, 32
N = H * W            # 2304 tokens per batch
NQ = N // 4          # 576 query tokens per core
NCORES = 8
SCALING = HD ** -0.5
LN_EPS = 1e-6
MT = N // 128        # 18 key tiles
MP = MT // 2         # 9 key-tile pairs
ROWS_Q = NQ // W     # 12 image rows per core
HALO = ROWS_Q + 4    # rows incl. conv halo
NHT = HALO * W       # 768 halo tokens
QCH = [(0, 512), (512, 64)]                                  # q chunks (bank)
NCH = [(0, 512), (512, 512), (1024, 512), (1536, 512), (2048, 256)]
HCH = [(0, 512), (512, 256)]
MSK_BUFS = 12
PP_BUFS = 6

_PROGS = {}
ABLATE = set()


def _bcast_ap(src, n=128):
    return bass.AP(tensor=src.tensor, offset=src.offset,
                   ap=[[0, n]] + src.ap[1:])


def _build_program(iters=1, qk_bias=False):
    nc = bacc.Bacc("TRN2", target_bir_lowering=False, debug=False,
                   enable_asserts=False, num_devices=NCORES)

    def din(name, shape, dt=F32):
        return nc.dram_tensor(name, shape, dt, kind="ExternalInput").ap()

    io = dict(
        wall=din("wall", [C, 6 * C], BF16),
        xall=din("xall", [C, N + NQ + NHT], BF16),
        sincos=din("sincos", [128, 2, N], BF16),
        f8c=din("f8c", [128, 2, 160], F8),
        ball=din("ball", [C, 31]),
        obk=din("obk", [1, 128], BF16),
        bval=din("bval", [1, C + NHT]),
        maskdr=din("maskdr", [MT, 64, 2, NH, NQ], F8),
        outT=nc.dram_tensor("outT", [C, NQ], F32, kind="ExternalOutput").ap(),
    )
    if qk_bias:
        io["rotkb"] = din("rotkb", [C, N], BF16)
        io["rotqb"] = din("rotqb", [C, NQ], BF16)
    with tile.TileContext(nc) as tc:
        with tc.tile_pool(name="persist", bufs=1) as P:
            for it in range(iters):
                _emit(nc, tc, P, io, qk_bias, it)
    nc.compile()
    return nc


def _emit(nc, tc, P, io, qk_bias, it=0):
    dma = nc.sync.dma_start

    def pt(tg, shape, dt=F32):
        return P.tile(shape, dt, tag=tg, name=f"{tg}_{it}")

    # ---- constants / inputs to SBUF ----
    # Inputs arrive as a handful of COALESCED DMAs (HWDGE dispatch is ~625ns
    # per DMA instruction) ordered by first use: weights -> x -> sin/cos ->
    # fp8 constants -> everything else.
    NT = N + NQ + NHT
    wall_sb = pt("wall_sb", [128, 2, 6, C], BF16)
    dma(wall_sb[:], bass.AP(tensor=io["wall"].tensor, offset=io["wall"].offset,
                            ap=[[6 * C, 128], [128 * 6 * C, 2], [1, 6 * C]]))
    w_sb = {nm: wall_sb[:, :, i, :]
            for i, nm in enumerate(("wq", "wqs", "wk", "wks", "wv", "wo"))}
    xall_sb = pt("xall_sb", [128, 2, NT], BF16)
    dma(xall_sb[:], bass.AP(tensor=io["xall"].tensor, offset=io["xall"].offset,
                            ap=[[NT, 128], [128 * NT, 2], [1, NT]]))
    xq_sb = xall_sb[:, :, 0:NQ]
    x_sb = xall_sb[:, :, NQ:NQ + N]
    xh_sb = xall_sb[:, :, NQ + N:NT]
    sc_sb = pt("sc_sb", [128, 2, N], BF16); dma(sc_sb[:], io["sincos"][:])
    sin_sb = sc_sb[:, 0, :]
    cos_sb = sc_sb[:, 1, :]
    f8c_sb = pt("f8c_sb", [128, 2, 160], F8); dma(f8c_sb[:], io["f8c"][:])
    id_sb = f8c_sb[0:64, :, 0:128]
    odr_sb = f8c_sb[:, :, 128:160]
    rb_sb_qk = {}
    if qk_bias:
        for nm, wd in (("rotkb", N), ("rotqb", NQ)):
            t = pt(f"{nm}_sb", [128, 2, wd], BF16)
            for kk in range(2):
                dma(t[:, kk, :], io[nm][128 * kk:128 * (kk + 1), :])
            rb_sb_qk[nm] = t
    ball_sb = pt("ball_sb", [128, 2, 31])
    dma(ball_sb[:], bass.AP(tensor=io["ball"].tensor, offset=io["ball"].offset,
                            ap=[[31, 128], [128 * 31, 2], [1, 31]]))
    b_sb = {nm: ball_sb[:, :, i:i + 1]
            for i, nm in enumerate(("bv", "bo", "lepe_b", "ln_g", "ln_b"))}
    lw_sb = ball_sb[:, :, 5:30]
    o1_sb = ball_sb[:, 0, 30:31]
    ok1_sb = pt("ok1_sb", [1, 128], BF16); dma(ok1_sb[:], io["obk"][0:1, :])
    bval_sb = pt("bval_sb", [128, C + NHT])
    dma(bval_sb[:], _bcast_ap(io["bval"]))
    bvr_sb = bval_sb[:, 0:C]
    val_sb = bval_sb[:, C:C + NHT]

    kr_bf = [pt(f"kr_bf{k}", [128, N], F8) for k in range(2)]
    qr_bf = [pt(f"qr_bf{k}", [128, NQ], F8) for k in range(2)]
    # DoubleRow repack: head j of the group at partitions 32j..32j+16,
    # plane i = head dims 16i..16(i+1).  Simple partition-slice DMAs — the
    # 2-level-partition AP form scrambles (HW-probed NaN).
    kr_dr = [pt(f"kr_dr{k}", [128, 2, N], F8) for k in range(2)]
    qr_dr = [pt(f"qr_dr{k}", [128, 2, NQ], F8) for k in range(2)]

    def dr_repack(dst, src):
        for j in range(4):
            for pl in range(2):
                dma(dst[32 * j:32 * j + 16, pl, :],
                    src[32 * j + 16 * pl:32 * j + 16 * (pl + 1), :])
    vn_f8 = pt("vn_f8", [128, MT, C], F8)
    vh_sb = [pt(f"vh_sb{k}", [128, NHT], BF16) for k in range(2)]
    vpad = [pt(f"vpad{k}", [128, HALO, W + 4], BF16) for k in range(2)]
    lepe_sb = [pt(f"lepe_sb{k}", [128, NQ], BF16) for k in range(2)]
    oat_sb = [pt(f"oat_sb{k}", [128, NQ], BF16) for k in range(2)]
    of_sb = [pt(f"of_sb{k}", [128, NQ], BF16) for k in range(2)]
    o2_sb = [pt(f"o2_sb{k}", [128, NQ], BF16) for k in range(2)]
    sq_sb = [pt(f"sq_sb{k}", [128, NQ], BF16) for k in range(2)]
    rb_sb = pt("rb_sb", [128, NQ])
    m1_sb = pt("m1_sb", [1, NQ], BF16)
    msq_sb = pt("msq_sb", [1, NQ])
    var_sb = pt("var_sb", [1, NQ])
    rstd_sb = pt("rstd_sb", [1, NQ])
    rstd8_sb = pt("rstd8_sb", [1, NQ], BF16)
    eps_sb = pt("eps_sb", [1, 1])
    nc.vector.memset(eps_sb[:], LN_EPS)
    outf_sb = [pt(f"outf_sb{k}", [128, NQ]) for k in range(2)]

    # msk/p/s pools open BEFORE phase 1 so their ranges don't alias the
    # phase-1 pools — mask DMAs prefetch from t~0 and the attention front
    # (inject/S/exp) runs concurrently with phase 1's tail.  The phase-1
    # PSUM pool is squeezed to 3 banks (bufs=1) so that sp(4)+stp(1)+pj(3)
    # fit in the 8 PSUM banks; the per-group op pool reuses pj's 3 banks
    # after phase 1 drains, with PP_BUFS pairs of exp output buffering the
    # attention front while the first AVs wait for those banks.
    mp_ctx = tc.tile_pool(name=f"mp_{it}", bufs=MSK_BUFS)
    pp_ctx = tc.tile_pool(name=f"pp_{it}", bufs=PP_BUFS)
    sp_ctx = tc.tile_pool(name=f"sp_{it}", bufs=2, space="PSUM")
    stp_ctx = tc.tile_pool(name=f"stp_{it}", bufs=1, space="PSUM")
    mp = mp_ctx.__enter__()
    pp = pp_ctx.__enter__()
    sp = sp_ctx.__enter__()
    stp = stp_ctx.__enter__()

    # ---- Phase 1: projections + rotary (swap folded into wqs/wks) ----
    with tc.tile_pool(name=f"pj_{it}", bufs=1, space="PSUM") as pj, \
         tc.tile_pool(name=f"pjs_{it}", bufs=2) as pjs:
        def proj_rot(jt):
            for (wn, wsn, dst, chunks, cols) in (
                    ("wq", "wqs", qr_bf[jt], QCH, NQ),
                    ("wk", "wks", kr_bf[jt], NCH, N)):
                for off, wd in chunks:
                    sl = slice(off, off + wd)
                    ps = pj.tile([128, 512], F32, tag="ps", name="ps")
                    pss = pj.tile([128, 512], F32, tag="pss", name="pss")
                    for kk in range(2):
                        nc.tensor.matmul(ps[:, :wd],
                                         w_sb[wn][:, kk, 128 * jt:128 * (jt + 1)],
                                         x_sb[:, kk, off:off + wd] if cols == N
                                         else xq_sb[:, kk, off:off + wd],
                                         start=(kk == 0), stop=(kk == 1))
                    for kk in range(2):
                        nc.tensor.matmul(pss[:, :wd],
                                         w_sb[wsn][:, kk, 128 * jt:128 * (jt + 1)],
                                         x_sb[:, kk, off:off + wd] if cols == N
                                         else xq_sb[:, kk, off:off + wd],
                                         start=(kk == 0), stop=(kk == 1))
                    tmp = pjs.tile([128, 512], BF16, tag="rt", name="rt")
                    tmp2 = pjs.tile([128, 512], BF16, tag="rt2", name="rt2")
                    nc.vector.tensor_tensor(tmp2[:, :wd], ps[:, :wd],
                                            cos_sb[:, sl], op=ALU.mult)
                    nc.vector.tensor_tensor(tmp[:, :wd], pss[:, :wd],
                                            sin_sb[:, sl], op=ALU.mult)
                    nc.vector.tensor_tensor(dst[:, sl], tmp2[:, :wd],
                                            tmp[:, :wd], op=ALU.add)
                    if qk_bias:
                        bt = rb_sb_qk["rotkb" if cols == N else "rotqb"]
                        nc.vector.tensor_tensor(dst[:, sl], dst[:, sl],
                                                bt[:, jt, sl], op=ALU.add)

        proj_rot(0)
        dr_repack(qr_dr[0], qr_bf[0])
        dr_repack(kr_dr[0], kr_bf[0])
        for mt in range(MT):
            ps = pj.tile([128, 256], F32, tag="psv", name="psv")
            for kk in range(2):
                nc.tensor.matmul(ps[:],
                                 x_sb[:, kk, 128 * mt:128 * (mt + 1)],
                                 w_sb["wv"][:, kk, :],
                                 start=(kk == 0), stop=(kk == 1))
            nc.vector.tensor_tensor(vn_f8[:, mt, :], ps[:], bvr_sb[:],
                                    op=ALU.add)
        proj_rot(1)
        dr_repack(qr_dr[1], qr_bf[1])
        dr_repack(kr_dr[1], kr_bf[1])
        for jt in range(2):
            for off, wd in HCH:
                ps = pj.tile([128, 512], F32, tag="ps", name="ps")
                for kk in range(2):
                    nc.tensor.matmul(ps[:, :wd],
                                     w_sb["wv"][:, kk, 128 * jt:128 * (jt + 1)],
                                     xh_sb[:, kk, off:off + wd],
                                     start=(kk == 0), stop=(kk == 1))
                # vT_halo = valid*bv + psum (keeps zero-padding exact)
                nc.vector.scalar_tensor_tensor(vh_sb[jt][:, off:off + wd],
                                               val_sb[:, off:off + wd],
                                               b_sb["bv"][:, jt, :],
                                               ps[:, :wd],
                                               op0=ALU.mult, op1=ALU.add)

    # ---- Phase 1b: depthwise 5x5 lepe conv (bf16 4x DVE taps) ----
    for jt in range(2):
        nc.gpsimd.memset(vpad[jt][:], 0.0)
        nc.gpsimd.tensor_copy(
            vpad[jt][:, :, 2:2 + W],
            vh_sb[jt][:].rearrange("p (r w) -> p r w", w=W))
        lp = lepe_sb[jt][:].rearrange("p (r w) -> p r w", w=W)
        first = True
        for dy in range(5):
            for dx in range(5):
                src = vpad[jt][:, dy:dy + ROWS_Q, dx:dx + W]
                wtap = lw_sb[:, jt, 5 * dy + dx:5 * dy + dx + 1]
                if first:
                    nc.vector.tensor_scalar(lp, src, wtap,
                                            b_sb["lepe_b"][:, jt, :],
                                            op0=ALU.mult, op1=ALU.add)
                    first = False
                else:
                    nc.vector.scalar_tensor_tensor(lp, src, wtap, lp,
                                                   op0=ALU.mult, op1=ALU.add)

    # ---- Phase 2: attention, two 4-head supergroups ----
    mio = io["maskdr"]
    for g in range(2):
        with tc.tile_pool(name=f"op{g}_{it}", bufs=1, space="PSUM") as op:
            o_main = op.tile([128, 512], F32, tag="o_main", name="o_main")
            den_main = op.tile([128, 512], F32, tag="den_main", name="den_main")
            od_stub = op.tile([128, 2, 64], F32, tag="od_stub", name="od_stub")
            if True:
                for tp in range(MP):
                    p_pair = pp.tile([128, 2, 4, NQ], F8, tag="p_sb",
                                     name="p_sb")
                    for half in range(2):
                        mt = 2 * tp + half
                        msk = mp.tile([64, 2, 4, NQ], F8, tag="msk",
                                      name="msk")
                        src = bass.AP(
                            tensor=mio.tensor,
                            offset=mio.offset + mt * (64 * 2 * NH * NQ)
                            + (4 * g) * NQ,
                            ap=[[2 * NH * NQ, 64], [NH * NQ, 2],
                                [NQ, 4], [1, NQ]])
                        if "maskdma" not in ABLATE:
                            dma(msk[:], src)
                        else:
                            nc.vector.memset(msk[:], 0.0)
                        s_pair = [sp.tile([128, 2, 512], F32, tag="s",
                                          name="s") for _ in range(2)]
                        stub = stp.tile([128, 4, 64], F32, tag="stub",
                                        name="stub")
                        for j in range(4):
                            pr, ln_ = j // 2, j % 2
                            nc.tensor.matmul(s_pair[pr][:, ln_, :], id_sb[:],
                                             msk[:, :, j, 0:512],
                                             start=True, stop=False,
                                             perf_mode=DR)
                            nc.tensor.matmul(stub[:, j, :], id_sb[:],
                                             msk[:, :, j, 512:576],
                                             start=True, stop=False,
                                             perf_mode=DR,
                                             skip_group_check=True)
                            lhs = kr_dr[g][32 * j:32 * j + 16, :,
                                           128 * mt:128 * (mt + 1)]
                            rq = qr_dr[g]
                            nc.tensor.matmul(s_pair[pr][:, ln_, :], lhs,
                                             rq[32 * j:32 * j + 16, :, 0:512],
                                             start=False, stop=True,
                                             tile_position=(32 * j, 0),
                                             perf_mode=DR)
                            nc.tensor.matmul(stub[:, j, :], lhs,
                                             rq[32 * j:32 * j + 16, :,
                                                512:576],
                                             start=False, stop=True,
                                             tile_position=(32 * j, 0),
                                             perf_mode=DR,
                                             skip_group_check=True)
                        for pr in range(2):
                            nc.scalar.activation(
                                p_pair[:, half, 2 * pr:2 * pr + 2, 0:512],
                                s_pair[pr][:], AF.Exp)
                        nc.scalar.activation(p_pair[:, half, :, 512:576],
                                             stub[:], AF.Exp)
                        # AV + den (fp8 data, non-DR — the ISA rejects
                        # DoubleRow with column tile packing)
                        ones32 = f8c_sb[:, 0, 128:160]
                        for j in range(4):
                            h = 4 * g + j
                            lhsv = vn_f8[:, mt, 32 * h:32 * (h + 1)]
                            nc.tensor.matmul(o_main[32 * j:32 * (j + 1), :],
                                             lhsv, p_pair[:, half, j, 0:512],
                                             start=(mt == 0), stop=(mt == MT - 1),
                                             tile_position=(0, 32 * j),
                                             skip_group_check=True)
                            nc.tensor.matmul(od_stub[32 * j:32 * (j + 1), 0, :],
                                             lhsv, p_pair[:, half, j, 512:576],
                                             start=(mt == 0), stop=(mt == MT - 1),
                                             tile_position=(0, 32 * j),
                                             skip_group_check=True)
                            nc.tensor.matmul(den_main[32 * j:32 * (j + 1), :],
                                             ones32, p_pair[:, half, j, 0:512],
                                             start=(mt == 0), stop=(mt == MT - 1),
                                             tile_position=(0, 32 * j),
                                             skip_group_check=True)
                            nc.tensor.matmul(od_stub[32 * j:32 * (j + 1), 1, :],
                                             ones32, p_pair[:, half, j, 512:576],
                                             start=(mt == 0), stop=(mt == MT - 1),
                                             tile_position=(0, 32 * j),
                                             skip_group_check=True)
            # normalize: the den matmuls already broadcast the key-sum to all
            # 32 partitions per head, so reciprocal reads PSUM directly
            nc.vector.reciprocal(rb_sb[:, 0:512], den_main[:])
            nc.vector.reciprocal(rb_sb[:, 512:576], od_stub[:, 1, :])
            nc.vector.tensor_tensor(oat_sb[g][:, 0:512], o_main[:],
                                    rb_sb[:, 0:512], op=ALU.mult)
            nc.vector.tensor_tensor(oat_sb[g][:, 512:576],
                                    od_stub[:, 0, :], rb_sb[:, 512:576],
                                    op=ALU.mult)
        nc.vector.tensor_tensor(of_sb[g][:], oat_sb[g][:], lepe_sb[g][:],
                                op=ALU.add)
    stp_ctx.__exit__(None, None, None)
    sp_ctx.__exit__(None, None, None)
    pp_ctx.__exit__(None, None, None)
    mp_ctx.__exit__(None, None, None)

    # ---- Phase 3: out-projection + residual + layernorm ----
    with tc.tile_pool(name=f"pwp_{it}", bufs=2, space="PSUM") as pwp:
        for jt in range(2):
            for off, wd in QCH:
                ps = pwp.tile([128, 512], F32, tag="pw", name="pw")
                for kk in range(2):
                    nc.tensor.matmul(ps[:, :wd],
                                     w_sb["wo"][:, kk, 128 * jt:128 * (jt + 1)],
                                     of_sb[kk][:, off:off + wd],
                                     start=(kk == 0), stop=(kk == 1))
                nc.vector.scalar_tensor_tensor(o2_sb[jt][:, off:off + wd],
                                               ps[:, :wd],
                                               b_sb["bo"][:, jt, :],
                                               xq_sb[:, jt, off:off + wd],
                                               op0=ALU.add, op1=ALU.add)
            nc.vector.tensor_tensor(sq_sb[jt][:], o2_sb[jt][:], o2_sb[jt][:],
                                    op=ALU.mult)
    with tc.tile_pool(name=f"stat_{it}", bufs=1, space="PSUM") as st:
        f8ones = f8c_sb[:, 0, 128:129]
        mu, ssq = {}, {}
        for off, wd in QCH:
            mu[off] = st.tile([1, wd], F32, tag=f"mu{off}", name="mu")
            ssq[off] = st.tile([1, wd], F32, tag=f"ssq{off}", name="ssq")
            for jt in range(2):
                nc.tensor.matmul(mu[off][:], f8ones,
                                 o2_sb[jt][:, off:off + wd],
                                 start=(jt == 0), stop=(jt == 1))
                nc.tensor.matmul(ssq[off][:], f8ones,
                                 sq_sb[jt][:, off:off + wd],
                                 start=(jt == 0), stop=(jt == 1))
        for off, wd in QCH:
            sl = slice(off, off + wd)
            nc.vector.tensor_scalar_mul(m1_sb[:, sl], mu[off][:], 1.0 / C)
            nc.vector.tensor_tensor(msq_sb[:, sl], m1_sb[:, sl],
                                    m1_sb[:, sl], op=ALU.mult)
            nc.vector.scalar_tensor_tensor(var_sb[:, sl], ssq[off][:],
                                           1.0 / C, msq_sb[:, sl],
                                           op0=ALU.mult, op1=ALU.subtract)
        nc.scalar.activation(rstd_sb[:], var_sb[:], AF.Sqrt, bias=eps_sb[:])
        with nc.allow_low_precision(reason="bf16 rstd; LN err ~4e-3"):
            nc.vector.reciprocal(rstd8_sb[:], rstd_sb[:])
    with tc.tile_pool(name=f"bc_{it}", bufs=1, space="PSUM") as bc:
        mb, rb = {}, {}
        for off, wd in QCH:
            mb[off] = bc.tile([128, wd], F32, tag=f"mb{off}", name="mb")
            rb[off] = bc.tile([128, wd], F32, tag=f"rb{off}", name="rb")
            nc.tensor.matmul(mb[off][:], ok1_sb[:], m1_sb[:, off:off + wd],
                             start=True, stop=True)
            nc.tensor.matmul(rb[off][:], ok1_sb[:], rstd8_sb[:, off:off + wd],
                             start=True, stop=True)
        for jt in range(2):
            for off, wd in QCH:
                sl = slice(off, off + wd)
                t1 = sq_sb[jt]  # scratch
                nc.vector.tensor_tensor(t1[:, sl], o2_sb[jt][:, sl],
                                        mb[off][:], op=ALU.subtract)
                nc.vector.tensor_tensor(t1[:, sl], t1[:, sl], rb[off][:],
                                        op=ALU.mult)
                nc.vector.affine_then_add(outf_sb[jt][:, sl], t1[:, sl],
                                          o2_sb[jt][:, sl],
                                          b_sb["ln_g"][:, jt, :],
                                          b_sb["ln_b"][:, jt, :])
            dma(io["outT"][128 * jt:128 * (jt + 1), :], outf_sb[jt][:])


def _host_inputs(x, sin, cos, mask, wq, bq, wk, bk, wv, bv,
                 lepe_w, lepe_b, wo, bo, ln_g, ln_b):
    bf = ml_dtypes.bfloat16
    f8 = ml_dtypes.float8_e4m3
    qk_bias = bool(np.abs(np.asarray(bq)).max() > 0
                   or np.abs(np.asarray(bk)).max() > 0)

    sinT = np.asarray(sin, np.float32).reshape(N, HD).T       # [32, N]
    cosT = np.asarray(cos, np.float32).reshape(N, HD).T
    sin128 = np.ascontiguousarray(np.tile(sinT, (4, 1))).astype(bf)
    cos128 = np.ascontiguousarray(np.tile(cosT, (4, 1))).astype(bf)

    # swap + sign-fold for the rotary partner projections:
    # shuf(k)[c] = sign(c) * k[c^1], sign = -1 for even c.
    swap_idx = np.arange(C) ^ 1
    sign = np.where(np.arange(C) % 2 == 0, -1.0, 1.0).astype(np.float32)
    wk_s = np.asarray(wk, np.float32) * SCALING
    bk_s = np.asarray(bk, np.float32) * SCALING
    wq_f = np.asarray(wq, np.float32)
    wks = wk_s[:, swap_idx] * sign[None, :]
    wqs = wq_f[:, swap_idx] * sign[None, :]

    wall = np.concatenate(
        [wq_f, wqs, wk_s, wks, np.asarray(wv, np.float32),
         np.asarray(wo, np.float32)], axis=1).astype(bf)        # [C, 6C]
    sincos = np.ascontiguousarray(
        np.stack([sin128, cos128], axis=1))                     # [128, 2, N]
    f8c = np.zeros((128, 2, 160), f8)
    f8c[np.arange(64), 0, np.arange(64)] = 1.0
    f8c[np.arange(64), 1, 64 + np.arange(64)] = 1.0
    f8c[:, :, 128:160] = 1.0
    ball = np.zeros((C, 31), np.float32)
    for i, a in enumerate((bv, bo, lepe_b, ln_g, ln_b)):
        ball[:, i] = np.asarray(a, np.float32)
    ball[:, 5:30] = np.asarray(lepe_w, np.float32).reshape(C, 25)
    ball[:, 30] = 1.0
    obk = np.ones((1, 128), np.float32)
    bv_row = np.asarray(bv, np.float32).reshape(1, C)
    common = dict(wall=wall, sincos=sincos, f8c=f8c, ball=ball,
                  obk=obk.astype(bf))
    rotqb_full = None
    if qk_bias:
        # rot(bias)[c, n] = b[c]*cos[c%32, n] + sign(c)*b[c^1]*sin[c%32, n]
        cos_full = np.tile(cosT, (8, 1))  # [256, N]
        sin_full = np.tile(sinT, (8, 1))
        bq_f = np.asarray(bq, np.float32)
        rotkb = (bk_s[:, None] * cos_full
                 + (sign * bk_s[swap_idx])[:, None] * sin_full)
        rotqb_full = (bq_f[:, None] * cos_full
                      + (sign * bq_f[swap_idx])[:, None] * sin_full
                      ).astype(bf)
        common["rotkb"] = rotkb.astype(bf)

    m8 = np.asarray(mask, np.float32).astype(f8)  # [NH, Nq, Nk]
    xs = np.asarray(x, np.float32).reshape(B, N, C)
    in_maps = []
    for c in range(NCORES):
        bb, qs = c // 4, c % 4
        xTb = np.ascontiguousarray(xs[bb].T)
        q0 = qs * NQ
        r0 = qs * ROWS_Q - 2
        halo = np.zeros((C, NHT), np.float32)
        vmask = np.zeros((1, NHT), np.float32)
        for r in range(HALO):
            ri = r0 + r
            if 0 <= ri < H:
                halo[:, r * W:(r + 1) * W] = xTb[:, ri * W:(ri + 1) * W]
                vmask[0, r * W:(r + 1) * W] = 1.0
        # maskdr[mt, p, i, h, q] = mask[h, q0+q, 128*mt + 64*i + p]
        mc = m8[:, q0:q0 + NQ, :]                     # [NH, NQ, Nk]
        mc = np.transpose(mc, (2, 0, 1))              # [Nk, NH, NQ]
        mc = mc.reshape(MT, 2, 64, NH, NQ)            # [mt, i, p, h, q]
        maskdr = np.ascontiguousarray(np.transpose(mc, (0, 2, 1, 3, 4)))
        m = dict(common)
        m.update(xall=np.ascontiguousarray(np.concatenate(
                     [xTb[:, q0:q0 + NQ], xTb, halo], axis=1)).astype(bf),
                 bval=np.concatenate([bv_row, vmask], axis=1),
                 maskdr=maskdr)
        if qk_bias:
            m["rotqb"] = np.ascontiguousarray(rotqb_full[:, q0:q0 + NQ])
        in_maps.append(m)
    return in_maps, qk_bias


def _get_prog(iters=1, qk_bias=False):
    key = (iters, qk_bias, tuple(sorted(ABLATE)))
    if key not in _PROGS:
        _PROGS[key] = _build_program(iters, qk_bias)
    return _PROGS[key]


def run_with_iters(in_maps, iters=1, qk_bias=False):
    nc = _get_prog(iters, qk_bias)
    return bass_utils.run_bass_kernel_spmd(nc, in_maps,
                                           core_ids=list(range(NCORES)))


def host_inputs(**inputs):
    in_maps, _ = _host_inputs(**inputs)
    return in_maps


def kernel(**inputs):
    in_maps, qk_bias = _host_inputs(**inputs)
    res = run_with_iters(in_maps, 1, qk_bias)
    out = np.empty((B, N, C), np.float32)
    for c in range(NCORES):
        bb, qs = c // 4, c % 4
        out[bb, qs * NQ:(qs + 1) * NQ, :] = res.results[c]["outT"].T
    return out.reshape(B, H, W, C)
